# revision 41
# baseline (speedup 1.0000x reference)
"""Trainium2 Bass kernel for EquivariantMultiheadAttention (sparse attention).

Problem shapes: b=4, n=512, c=256, h=8, d=32, dg=6, hid=16.

Strategy (8 NeuronCores, no collectives):
  - Shard (batch b x n-half): core i handles b = i//2, query rows
    n0 = 256*(i%2) .. n0+256.  Keys are mask-compacted per batch to
    m_pad (=272 for this seed); padded columns get a -1e38 bias.
  - ACT (scalar engine) is the bottleneck (~157us busy): the two
    location-MLP Silu passes cover NP*m_pad elements per lane per layer
    at 0.83ns/elem plus ~185ns per instruction.  To amortize that, keys
    split into [0,256) (main pipeline) + [256,m_pad) (a "c1" prephase
    reusing the same PSUM tags), and the main pipeline runs 6/6/4-row
    steps ([128,6,256] PSUM tiles = 3 banks, 2 bufs), software-pipelined
    with a one-step skew (L1(i) | silu1(i) | L2(i-1) | silu2(i-1) |
    L3(i-2)) so ACT never waits on the L2 latency chain.
  - L3 accumulates a GROUP PAIR into one PSUM bank ([0:256)/[256:512))
    so one silu3 covers 32 rows; pre is seeded with A_feat (copy from
    PSUM) and the pair of A_loc rows is added in one DVE op.
  - Softmax: no max-subtraction (a constant -4 bias keeps exp in f16
    range; padding exp(-1e38)=0).  exp emits row sums via accum_out;
    normalization is folded into the att transpose by multiplying with
    diag(1/s) built per group on DVE from an f16 identity.
  - AV accumulates m-chunks into one PSUM tile per 4-group block; the
    output projection + store for rows 0:128 completes mid-exp-phase,
    leaving only the last block's chain on the tail.
  - All heavy matmuls are f32r with >=256 output columns (1 cycle/row);
    AV/transposes run in f16.  A PE p-state warmup (dummy matmuls at
    t~0) and an early dummy Silu (pulls the ACT table load into the
    initial DMA wait) trim the startup; DMAs are spread across the
    SP/HWDGE and Pool/SWDGE queues with critical constants first.
"""
import sys

sys.path.insert(0, "/opt/trn_rl_repo")

import numpy as np
import concourse.bacc as bacc
import concourse.mybir as mybir
import concourse.tile as tile
from concourse.bass_utils import run_bass_kernel_spmd

F32 = mybir.dt.float32
F32R = mybir.dt.float32r
F16 = mybir.dt.float16
AF = mybir.ActivationFunctionType

B, N, C, H, DG, HID = 4, 512, 256, 8, 6, 16
D = C // H          # 32
NP = N // 2         # 256 query rows per core
NG = NP // 16       # 16 groups of 16 rows
MC0 = 256           # main key chunk
NEG = np.float32(-1e38)
AV_DT = F16


def _build(nc_mod, m_pad):
    nc = nc_mod
    M = m_pad
    MC1 = M - MC0               # tail key chunk (16 for this seed)
    assert MC1 >= 1 and 1024 % MC1 == 0 and NP % (1024 // MC1) == 0
    C1R = 1024 // MC1           # rows per c1 step (64)
    C1S = NP // C1R             # c1 steps (4)
    SUB = 512 // MC1            # rows per c1 matmul (32)
    MT = [(t, min(128, M - 128 * t)) for t in range((M + 127) // 128)]

    # ---------------- I/O ----------------
    gt = nc.declare_dram_parameter("gt", [DG, NP, MC0], F32, isOutput=False)
    gtc1 = nc.declare_dram_parameter("gtc1", [DG, NP, MC1], F32, isOutput=False)
    ctq = nc.declare_dram_parameter("ctq", [C, NP], F32, isOutput=False)
    ctk = nc.declare_dram_parameter("ctk", [C, M], F32, isOutput=False)
    wq = nc.declare_dram_parameter("wq", [C, C], F32, isOutput=False)
    wk = nc.declare_dram_parameter("wk", [C, C], F32, isOutput=False)
    win = nc.declare_dram_parameter("win", [C, C], F32, isOutput=False)
    wout = nc.declare_dram_parameter("wout", [C, C], F32, isOutput=False)
    bq = nc.declare_dram_parameter("bq", [1, C], F32, isOutput=False)
    bk = nc.declare_dram_parameter("bk", [1, C], F32, isOutput=False)
    bin_ = nc.declare_dram_parameter("bin", [1, C], F32, isOutput=False)
    bout = nc.declare_dram_parameter("bout", [1, C], F32, isOutput=False)
    w1 = nc.declare_dram_parameter("w1", [DG, 128], F32, isOutput=False)
    w2 = nc.declare_dram_parameter("w2", [128, 128], F32, isOutput=False)
    w3 = nc.declare_dram_parameter("w3", [128, 8], F32, isOutput=False)
    b1 = nc.declare_dram_parameter("b1", [128, 1], F32, isOutput=False)
    b2 = nc.declare_dram_parameter("b2", [128, 1], F32, isOutput=False)
    b3 = nc.declare_dram_parameter("b3", [128, 1], F32, isOutput=False)
    mb = nc.declare_dram_parameter("mb", [1, M], F32, isOutput=False)
    onesc = nc.declare_dram_parameter("onesc", [1, 128], F32, isOutput=False)
    zeros = nc.declare_dram_parameter("zeros", [128, 128], F32, isOutput=False)
    identh = nc.declare_dram_parameter("identh", [128, 128], AV_DT,
                                       isOutput=False)
    out = nc.declare_dram_parameter("out", [NP, C], F32, isOutput=True)

    with tile.TileContext(nc) as tc:
        import contextlib
        with contextlib.ExitStack() as ctx:
            cst = ctx.enter_context(tc.tile_pool(name="cst", bufs=1))
            big = ctx.enter_context(tc.tile_pool(name="big", bufs=1))
            gtp = ctx.enter_context(tc.tile_pool(name="gtp", bufs=2))
            xp = ctx.enter_context(tc.tile_pool(name="xp", bufs=2))
            smp = ctx.enter_context(tc.tile_pool(name="smp", bufs=3))
            pmm = ctx.enter_context(tc.tile_pool(name="pmm", bufs=2,
                                                 space="PSUM"))
            pl3 = ctx.enter_context(tc.tile_pool(name="pl3", bufs=1,
                                                 space="PSUM"))
            pms = ctx.enter_context(tc.tile_pool(name="pms", bufs=1,
                                                 space="PSUM"))

            # ---- critical-path constants on the SP/HWDGE queue ----
            gtc1_sb = cst.tile([DG, NP, MC1], F32R, tag="gtc1")
            nc.sync.dma_start(out=gtc1_sb, in_=gtc1[:, :, :].bitcast(F32R))
            w1_sb = cst.tile([DG, 128], F32R, tag="w1")
            nc.sync.dma_start(out=w1_sb, in_=w1[:, :].bitcast(F32R))
            b1_sb = cst.tile([128, 1], F32, tag="b1")
            nc.sync.dma_start(out=b1_sb, in_=b1[:, :])
            w2_sb = cst.tile([128, 128], F32R, tag="w2")
            nc.sync.dma_start(out=w2_sb, in_=w2[:, :].bitcast(F32R))
            b2_sb = cst.tile([128, 1], F32, tag="b2")
            nc.sync.dma_start(out=b2_sb, in_=b2[:, :])
            b3_sb = cst.tile([128, 1], F32, tag="b3")
            nc.sync.dma_start(out=b3_sb, in_=b3[:, :])

            gt_tiles = {}

            def fetch_gt(g):
                t = gtp.tile([DG, 16, MC0], F32R, tag="gt", name=f"gt{g}")
                nc.sync.dma_start(
                    out=t, in_=gt[:, 16 * g:16 * (g + 1), :].bitcast(F32R))
                gt_tiles[g] = t

            fetch_gt(0)
            fetch_gt(1)

            # ---- early constants on the DVE queue ----
            onr = cst.tile([1, 512], F32R, tag="onr")
            nc.vector.memset(onr.bitcast(F32), 1.0)
            # dummy silu: pulls the Silu table load to t~0 (hidden in the
            # initial DMA wait) so the first real silu pays no load
            scrap = cst.tile([1, 1], F32, tag="scrap")
            nc.scalar.activation(out=scrap, in_=b1_sb[:1, :], func=AF.Silu,
                                 bias=b1_sb[:1, :], scale=1.0)
            # PE p-state warmup: tiny dummy matmuls start the clock-ramp
            # timer while the critical DMAs are still in flight
            pwarm = pms.tile([128, 512], F32, tag="ms", name="pwarm")
            for _ in range(3):
                nc.tensor.matmul(pwarm[:, :128], onr[:, :128],
                                 onr[:, :128], start=True, stop=True)
            wq_sb, wk_sb, ctq_sb, ctk_sb = [], [], [], []
            for ci in range(2):
                t = cst.tile([128, C], F32R, tag=f"wq{ci}", name=f"wq{ci}")
                nc.sync.dma_start(
                    out=t, in_=wq[128 * ci:128 * (ci + 1), :].bitcast(F32R))
                wq_sb.append(t)
                t = cst.tile([128, NP], F32R, tag=f"cq{ci}", name=f"cq{ci}")
                nc.sync.dma_start(
                    out=t, in_=ctq[128 * ci:128 * (ci + 1), :].bitcast(F32R))
                ctq_sb.append(t)
            bq_sb = cst.tile([1, C], F32R, tag="bq")
            nc.sync.dma_start(out=bq_sb, in_=bq[:, :].bitcast(F32R))

            # ---- Pool (SWDGE) queue: w3 variant sources ----
            zs = cst.tile([128, 128], F32R, tag="zs")
            nc.gpsimd.dma_start(out=zs, in_=zeros[:, :].bitcast(F32R))
            w3c = cst.tile([128, 8], F32R, tag="w3c")
            nc.gpsimd.dma_start(out=w3c, in_=w3[:, :].bitcast(F32R))
            w3_sb = [cst.tile([128, 128], F32R, tag=f"w3{j}", name=f"w3{j}")
                     for j in range(16)]

            def build_w3(j):
                nc.gpsimd.tensor_copy(out=w3_sb[j], in_=zs)
                nc.gpsimd.tensor_copy(out=w3_sb[j][:, 8 * j:8 * j + 8],
                                      in_=w3c)

            # ---- persistent SBUF tensors ----
            qt_sb = [big.tile([128, NP], F32R, tag=f"qt{i}", name=f"qt{i}")
                     for i in range(2)]
            kt_sb = [big.tile([128, M], F32R, tag=f"kt{i}", name=f"kt{i}")
                     for i in range(2)]
            qm_all = [big.tile([128, NP, 8], F32R, tag=f"qm{i}",
                               name=f"qm{i}") for i in range(2)]
            x2c1 = big.tile([128, NP, MC1], F32R, tag="x2c1")
            aloc = big.tile([128, NG, M], F32, tag="aloc")
            pre = big.tile([128, NG, M], F32, tag="pre")
            attT = big.tile([128, len(MT), NG, 16, 8], AV_DT, tag="attT")
            v_sb = [big.tile([128, C], AV_DT, tag=f"v{t}", name=f"v{t}")
                    for t, _ in MT]
            win_sb, wout_sb = [], []
            misc = {}

            # ---- deferred-prologue emitters ----
            def emit_dve_batch2():
                for ci in range(2):
                    t = cst.tile([128, C], F32R, tag=f"wk{ci}", name=f"wk{ci}")
                    nc.sync.dma_start(
                        out=t, in_=wk[128 * ci:128 * (ci + 1), :].bitcast(F32R))
                    wk_sb.append(t)
                    t = cst.tile([128, M], F32R, tag=f"ck{ci}", name=f"ck{ci}")
                    nc.sync.dma_start(
                        out=t, in_=ctk[128 * ci:128 * (ci + 1), :].bitcast(F32R))
                    ctk_sb.append(t)
                t = cst.tile([1, C], F32R, tag="bk", name="bk_sb")
                nc.sync.dma_start(out=t, in_=bk[:, :].bitcast(F32R))
                misc["bk"] = t
                t = cst.tile([1, M], F32R, tag="mb", name="mb_sb")
                nc.sync.dma_start(out=t, in_=mb[:, :].bitcast(F32R))
                misc["mb"] = t
                t = cst.tile([1, 128], F32R, tag="onc", name="onc_sb")
                nc.sync.dma_start(out=t, in_=onesc[:, :].bitcast(F32R))
                misc["onc"] = t

            def proj_T(dst, w_tiles, b_row, rhs_tiles, nfree, nm, ct):
                p = pms.tile([128, 512], F32, tag="ms",
                             name=f"pj_{nm}{ct}")
                for ci in range(2):
                    nc.tensor.matmul(
                        p[:, :nfree],
                        w_tiles[ci][:, 128 * ct:128 * (ct + 1)],
                        rhs_tiles[ci][:, :nfree],
                        start=(ci == 0), stop=False)
                nc.tensor.matmul(
                    p[:, :nfree], b_row[:, 128 * ct:128 * (ct + 1)],
                    onr[:, :nfree], start=False, stop=True)
                nc.vector.tensor_copy(out=dst[ct], in_=p[:, :nfree])

            def emit_qt(ct):
                proj_T(qt_sb, wq_sb, bq_sb, ctq_sb, NP, "q", ct)

            def emit_kt(ct):
                proj_T(kt_sb, wk_sb, misc["bk"], ctk_sb, M, "k", ct)

            def emit_qm_zero():
                nc.gpsimd.memset(qm_all[0].bitcast(F32), 0.0)
                nc.gpsimd.memset(qm_all[1].bitcast(F32), 0.0)

            def emit_qm_fill():
                for h in range(H):
                    a = 32 * (h % 4)
                    nc.vector.tensor_copy(
                        out=qm_all[h // 4][a:a + 32, :, h],
                        in_=qt_sb[h // 4][a:a + 32, :])

            def emit_v_consts():
                for ci in range(2):
                    t = cst.tile([128, C], F32R, tag=f"wi{ci}", name=f"wi{ci}")
                    nc.gpsimd.dma_start(
                        out=t, in_=win[128 * ci:128 * (ci + 1), :].bitcast(F32R))
                    win_sb.append(t)
                t = cst.tile([1, C], F32R, tag="bi", name="bi_sb")
                nc.gpsimd.dma_start(out=t, in_=bin_[:, :].bitcast(F32R))
                misc["bi"] = t

            def emit_v(t_, msz):
                p = pms.tile([128, 512], F32, tag="ms", name=f"pv{t_}")
                for ci in range(2):
                    nc.tensor.matmul(
                        p[:msz, :C],
                        ctk_sb[ci][:, 128 * t_:128 * t_ + msz],
                        win_sb[ci], start=(ci == 0), stop=False)
                nc.tensor.matmul(
                    p[:msz, :C], misc["onc"][:, :msz],
                    misc["bi"], start=False, stop=True)
                nc.vector.tensor_copy(out=v_sb[t_][:msz, :],
                                      in_=p[:msz, :C])

            def emit_tail_consts():
                for ci in range(2):
                    t = cst.tile([128, C], F32R, tag=f"wo{ci}", name=f"wo{ci}")
                    nc.gpsimd.dma_start(
                        out=t,
                        in_=wout[128 * ci:128 * (ci + 1), :].bitcast(F32R))
                    wout_sb.append(t)
                t = cst.tile([1, C], F32R, tag="bo", name="bo_sb")
                nc.gpsimd.dma_start(out=t, in_=bout[:, :].bitcast(F32R))
                misc["bo"] = t
                t = cst.tile([128, 128], AV_DT, tag="idh", name="idh_sb")
                nc.gpsimd.dma_start(out=t, in_=identh[:, :])
                misc["idh"] = t

            # ---------------- c1 prephase (keys [256, m_pad)) -----------
            x2c1v = x2c1.rearrange("p (g r) k -> p g r k", g=NG)
            x1cs = {}

            def emit_l2c1(s):
                p2c = pmm.tile([128, C1R, MC1], F32, tag="mm",
                               name=f"p2c{s}")
                for j in range(0, C1R, SUB):
                    nc.tensor.matmul(p2c[:, j:j + SUB, :], w2_sb,
                                     x1cs[s][:, j:j + SUB, :],
                                     start=True, stop=True)
                nc.scalar.activation(
                    out=x2c1[:, s * C1R:(s + 1) * C1R, :], in_=p2c,
                    func=AF.Silu, bias=b2_sb, scale=1.0)

            for s in range(C1S):
                if s == 0:
                    emit_dve_batch2()
                for j in range(4):
                    build_w3(4 * s + j)
                p1c = pmm.tile([128, C1R, MC1], F32, tag="mm",
                               name=f"p1c{s}")
                for j in range(0, C1R, SUB):
                    nc.tensor.matmul(
                        p1c[:, j:j + SUB, :], w1_sb,
                        gtc1_sb[:, s * C1R + j:s * C1R + j + SUB, :],
                        start=True, stop=True)
                x1c = xp.tile([128, C1R, MC1], F32R, tag="x1",
                              name=f"x1c{s}")
                nc.scalar.activation(out=x1c, in_=p1c, func=AF.Silu,
                                     bias=b1_sb, scale=1.0)
                x1cs[s] = x1c
                if s >= 1:
                    emit_l2c1(s - 1)
                if s == 1:
                    emit_qt(0)
                    emit_v_consts()
                if s == 2:
                    emit_qt(1)
                if s == 3:
                    emit_kt(0)
                    emit_qm_zero()
                    emit_v(*MT[0])
            emit_l2c1(C1S - 1)

            def emit_l3c1():
                pc1 = pl3.tile([128, 512], F32, tag="l3", name="pc1")
                pc1v = pc1[:, :NG * MC1].rearrange("p (g k) -> p g k", g=NG)
                for r in range(16):
                    nc.tensor.matmul(pc1v, w3_sb[r], x2c1v[:, :, r, :],
                                     start=(r == 0), stop=(r == 15))
                nc.scalar.activation(out=aloc[:, :, MC0:], in_=pc1v,
                                     func=AF.Silu, bias=b3_sb, scale=1.0)

            # ---------------- main pipeline: 64 steps of 4 rows ----------
            deferred = {
                0: [lambda: emit_kt(1), lambda: emit_v(*MT[1])],
                1: ([emit_qm_fill, lambda: emit_v(*MT[2])] if len(MT) > 2
                    else [emit_qm_fill]),
                4: [emit_tail_consts],
            }
            STEPS = [(g, r0, nr) for g in range(NG)
                     for r0, nr in ((0, 6), (6, 6), (12, 4))]
            NS = len(STEPS)
            p3s = {}
            pafs = {}
            x1s = {}
            x2s = {}

            def emit_l2(k):
                # L2 of step k, one step behind L1 (keeps ACT off the
                # L2 latency chain)
                gk, rk, nk = STEPS[k]
                p2 = pmm.tile([128, 6, MC0], F32, tag="mm", name=f"p2_{k}")
                for j in range(nk):
                    nc.tensor.matmul(p2[:, j, :], w2_sb, x1s[k][:, j, :],
                                     start=True, stop=True)
                x2 = xp.tile([128, 6, MC0], F32R, tag="x2", name=f"x2_{k}")
                nc.scalar.activation(out=x2[:, :nk, :], in_=p2[:, :nk, :],
                                     func=AF.Silu, bias=b2_sb, scale=1.0)
                x2s[k] = x2
                del x1s[k]

            def emit_l3(k):
                # L3 of a group pair accumulates into one bank: even group
                # in [0:256], odd group in [256:512]; one silu3 covers both.
                gk, rk, nk = STEPS[k]
                q = gk // 2
                if rk == 0 and gk % 2 == 0:
                    p3s[q] = pl3.tile([128, 2, MC0], F32, tag="l3",
                                      name=f"p3_{q}")
                for j in range(nk):
                    nc.tensor.matmul(p3s[q][:, gk % 2, :], w3_sb[rk + j],
                                     x2s[k][:, j, :],
                                     start=(rk + j == 0), stop=(rk + j == 15))
                del x2s[k]
                if rk + nk == 16:
                    # pre := paf as soon as A_feat lands (per group)
                    nc.vector.tensor_copy(out=pre[:, gk, :],
                                          in_=pafs.pop(gk)[:, :M])
                    if gk % 2 == 1:
                        nc.scalar.activation(
                            out=aloc[:, 2 * q:2 * q + 2, :MC0],
                            in_=p3s[q], func=AF.Silu, bias=b3_sb, scale=1.0)
                        nc.vector.tensor_add(
                            pre[:, 2 * q:2 * q + 2, :],
                            pre[:, 2 * q:2 * q + 2, :],
                            aloc[:, 2 * q:2 * q + 2, :])

            for i in range(NS):
                g, r0, nr = STEPS[i]
                if r0 == 0 and g + 2 < NG:
                    fetch_gt(g + 2)
                p1 = pmm.tile([128, 6, MC0], F32, tag="mm", name=f"p1_{i}")
                for j in range(nr):
                    nc.tensor.matmul(p1[:, j, :], w1_sb,
                                     gt_tiles[g][:, r0 + j, :],
                                     start=True, stop=True)
                x1 = xp.tile([128, 6, MC0], F32R, tag="x1", name=f"x1_{i}")
                nc.scalar.activation(out=x1[:, :nr, :], in_=p1[:, :nr, :],
                                     func=AF.Silu, bias=b1_sb, scale=1.0)
                x1s[i] = x1
                if i == 0:
                    emit_l3c1()
                if i >= 2:
                    emit_l3(i - 2)
                if i >= 1:
                    emit_l2(i - 1)
                if (r0 == 6 and g > 0) or (r0 == 12 and g == 0):
                    paf = pms.tile([128, 512], F32, tag="ms", name=f"paf{g}")
                    nc.tensor.matmul(paf[:, :M],
                                     qm_all[0][:, 16 * g:16 * g + 16, :],
                                     kt_sb[0], start=True, stop=False)
                    nc.tensor.matmul(paf[:, :M],
                                     qm_all[1][:, 16 * g:16 * g + 16, :],
                                     kt_sb[1], start=False, stop=False)
                    nc.tensor.matmul(paf[:, :M], misc["onc"], misc["mb"],
                                     start=False, stop=True)
                    pafs[g] = paf
                for fn in deferred.get(i, ()):
                    fn()
            emit_l2(NS - 1)
            emit_l3(NS - 2)
            emit_l3(NS - 1)

            # ---------------- P2: exp softmax + scaled transpose ---------
            # bneg4 = -4.0, built from the LAST silu3 output so the Tile
            # scheduler cannot hoist any Exp (different ACT table) into the
            # Silu stream.
            bneg4 = cst.tile([128, 1], F32, tag="bneg4")
            nc.scalar.activation(out=bneg4, in_=aloc[:, NG - 1, :1],
                                 func=AF.Copy, bias=-4.0, scale=0.0)
            pav = pmm.tile([128, 4, MC0], F32, tag="mm", name="pav")
            pavv = pav.rearrange("p a b -> p (a b)")[:, :2 * C].rearrange(
                "p (a b) -> p a b", a=2)

            avf = smp.tile([128, 2, C], F32R, tag="avf")
            po_t = {}

            def emit_av(g_lo, g_hi):
                w = 16 * (g_hi - g_lo + 1)
                for h in range(H):
                    for t_, msz in MT:
                        nc.tensor.matmul(
                            pavv[32 * (h % 4):32 * (h % 4) + 32, h // 4,
                                 16 * g_lo:16 * g_lo + w],
                            v_sb[t_][:msz, 32 * h:32 * h + 32],
                            attT[:msz, t_, g_lo:g_hi + 1, :, h],
                            start=(t_ == 0),
                            stop=(t_ == len(MT) - 1),
                            tile_position=(0, 32 * (h % 4)))

            def emit_avf(nt, eng):
                eng(out=avf[:, :, 128 * nt:128 * (nt + 1)],
                    in_=pavv[:, :, 128 * nt:128 * (nt + 1)])

            def emit_po(nt):
                po = pmm.tile([128, 6, MC0], F32, tag="mm", name=f"po{nt}")
                for ci in range(2):
                    nc.tensor.matmul(
                        po[:, 0, :C], avf[:, ci, 128 * nt:128 * (nt + 1)],
                        wout_sb[ci], start=(ci == 0), stop=False)
                nc.tensor.matmul(po[:, 0, :C], misc["onc"],
                                 misc["bo"], start=False, stop=True)
                po_t[nt] = po

            def emit_osb(nt, eng):
                o_sb = smp.tile([128, C], F32, tag="osb", name=f"osb{nt}")
                eng(out=o_sb, in_=po_t[nt][:, 0, :C])
                nc.sync.dma_start(out=out[128 * nt:128 * (nt + 1), :],
                                  in_=o_sb)

            for g in range(NG):
                att = smp.tile([128, M], AV_DT, tag="att", name=f"att{g}",
                               bufs=6)
                sm = smp.tile([128, 1], F32, tag="sm", name=f"sm{g}", bufs=6)
                nc.scalar.activation(out=att, in_=pre[:, g, :], func=AF.Exp,
                                     bias=bneg4, scale=1.0, accum_out=sm)
                rc = smp.tile([128, 1], F32, tag="rc", name=f"rc{g}", bufs=6)
                nc.vector.reciprocal(out=rc, in_=sm)
                dg_t = smp.tile([128, 128], AV_DT, tag="dg", name=f"dg{g}",
                                bufs=6)
                nc.vector.tensor_scalar_mul(dg_t, misc["idh"], rc)
                pT = (pms if g % 2 == 0 else pl3).tile(
                    [128, 512], F32, tag=("ms" if g % 2 == 0 else "l3"),
                    name=f"pT{g}")
                pT3 = pT[:, :len(MT) * 128].rearrange("p (t q) -> p t q",
                                                      t=len(MT))
                pT3r = pT[:, :len(MT) * 128].rearrange(
                    "p (t r h) -> p t r h", t=len(MT), r=16)
                for t_, msz in MT:
                    nc.tensor.matmul(pT3[:msz, t_, :],
                                     att[:, 128 * t_:128 * t_ + msz],
                                     dg_t, start=True, stop=True)
                if g % 4 == 1:
                    nc.scalar.copy(out=attT[:, :, g, :, :], in_=pT3r)
                else:
                    nc.vector.tensor_copy(out=attT[:, :, g, :, :], in_=pT3r)
                if g == 3:
                    emit_av(0, 3)
                elif g == 7:
                    emit_av(4, 7)
                elif g == 8:
                    emit_avf(0, nc.vector.tensor_copy)
                elif g == 9:
                    emit_po(0)
                elif g == 10:
                    emit_osb(0, nc.vector.tensor_copy)
                elif g == 11:
                    emit_av(8, 11)
                elif g == 14:
                    emit_av(12, 14)
                elif g == 15:
                    emit_av(15, 15)

            # ---------------- P3: finish rows 128..255 -------------------
            emit_avf(1, nc.vector.tensor_copy)
            emit_po(1)
            emit_osb(1, nc.scalar.copy)

    nc.finalize()
    return nc


_CACHE = {}


def _get_nc(m_pad):
    if m_pad not in _CACHE:
        _CACHE[m_pad] = _build(bacc.Bacc(None, target_bir_lowering=False), m_pad)
    return _CACHE[m_pad]


def prepare(inputs):
    """Host-side sharding/packing. Returns (nc, in_maps, assemble)."""
    pg = np.asarray(inputs["pairwise_g"], np.float32)
    cf = np.asarray(inputs["coset_functions"], np.float32)
    mask = np.asarray(inputs["mask"])
    idxs = [np.where(mask[b])[0] for b in range(B)]
    maxc = max(len(ix) for ix in idxs)
    m_pad = max(MC0 + 16, -(-maxc // 16) * 16)

    w1a = np.ascontiguousarray(
        np.asarray(inputs["loc_w1"], np.float32).transpose(1, 0, 2).reshape(
            DG, 128))
    w2b = np.zeros((128, 128), np.float32)
    lw2 = np.asarray(inputs["loc_w2"], np.float32)
    for h in range(H):
        w2b[16 * h:16 * (h + 1), 16 * h:16 * (h + 1)] = lw2[h]
    lw3 = np.asarray(inputs["loc_w3"], np.float32)
    w3p = np.zeros((128, 8), np.float32)
    for h in range(H):
        w3p[16 * h:16 * (h + 1), h] = lw3[h, :, 0]
    b1v = np.asarray(inputs["loc_b1"], np.float32).reshape(128, 1)
    b2v = np.asarray(inputs["loc_b2"], np.float32).reshape(128, 1)
    b3v = np.tile(np.asarray(inputs["loc_b3"], np.float32).reshape(8), 16)
    b3v = b3v.reshape(128, 1)

    idh = np.eye(128, dtype=np.float16)

    common = {
        "wq": np.asarray(inputs["fc_q_w"], np.float32) / np.float32(16.0),
        "wk": np.asarray(inputs["fc_k_w"], np.float32),
        "win": np.asarray(inputs["in_w"], np.float32),
        "wout": np.asarray(inputs["out_w"], np.float32),
        "bq": (np.asarray(inputs["fc_q_b"], np.float32) / np.float32(16.0)
               ).reshape(1, C),
        "bk": np.asarray(inputs["fc_k_b"], np.float32).reshape(1, C),
        "bin": np.asarray(inputs["in_b"], np.float32).reshape(1, C),
        "bout": np.asarray(inputs["out_b"], np.float32).reshape(1, C),
        "w1": w1a, "w2": w2b, "w3": w3p,
        "b1": b1v, "b2": b2v, "b3": b3v,
        "onesc": np.ones((1, 128), np.float32),
        "zeros": np.zeros((128, 128), np.float32),
        "identh": idh,
    }
    common = {k: np.ascontiguousarray(v) for k, v in common.items()}

    in_maps = []
    for core in range(8):
        b, nh = core // 2, core % 2
        ix = idxs[b]
        cnt = len(ix)
        n0 = NP * nh
        gtb = np.zeros((DG, NP, m_pad), np.float32)
        gtb[:, :, :cnt] = pg[b, n0:n0 + NP][:, ix, :].transpose(2, 0, 1)
        ctkb = np.zeros((C, m_pad), np.float32)
        ctkb[:, :cnt] = cf[b, ix, :].T
        mbv = np.zeros((1, m_pad), np.float32)
        mbv[0, cnt:] = NEG
        im = dict(common)
        im["gt"] = np.ascontiguousarray(gtb[:, :, :MC0])
        im["gtc1"] = np.ascontiguousarray(gtb[:, :, MC0:])
        im["ctq"] = np.ascontiguousarray(cf[b, n0:n0 + NP, :].T)
        im["ctk"] = ctkb
        im["mb"] = mbv
        in_maps.append(im)

    def assemble(results):
        o = np.empty((B, N, C), np.float32)
        for core in range(8):
            b, nh = core // 2, core % 2
            o[b, NP * nh:NP * (nh + 1), :] = results[core]["out"]
        return o

    return _get_nc(m_pad), in_maps, assemble


def kernel(**inputs) -> np.ndarray:
    nc, in_maps, assemble = prepare(inputs)
    res = run_bass_kernel_spmd(nc, in_maps, list(range(8)))
    return assemble(res.results)


# revision 42
# speedup vs baseline: 1.0586x; 1.0586x over previous
"""Trainium2 Bass kernel for EquivariantMultiheadAttention (sparse attention).

Problem shapes: b=4, n=512, c=256, h=8, d=32, dg=6, hid=16.

Strategy (8 NeuronCores, no collectives):
  - Shard (batch b x n-half): core i handles b = i//2, query rows
    n0 = 256*(i%2) .. n0+256.  Keys are mask-compacted per batch to
    m_pad (=272 for this seed); padded columns get a -1e38 bias.
  - ACT (scalar engine) is the bottleneck (~157us busy): the two
    location-MLP Silu passes cover NP*m_pad elements per lane per layer
    at 0.83ns/elem plus ~185ns per instruction.  To amortize that, keys
    split into [0,256) (main pipeline) + [256,m_pad) (a "c1" prephase
    reusing the same PSUM tags), and the main pipeline runs 6/6/4-row
    steps ([128,6,256] PSUM tiles = 3 banks, 2 bufs), software-pipelined
    with a one-step skew (L1(i) | silu1(i) | L2(i-1) | silu2(i-1) |
    L3(i-2)) so ACT never waits on the L2 latency chain.
  - L3 accumulates a GROUP PAIR into one PSUM bank ([0:256)/[256:512))
    so one silu3 covers 32 rows; pre is seeded with A_feat (copy from
    PSUM) and the pair of A_loc rows is added in one DVE op.
  - Softmax: no max-subtraction (a constant -4 bias keeps exp in f16
    range; padding exp(-1e38)=0).  exp emits row sums via accum_out;
    normalization is folded into the att transpose by multiplying with
    diag(1/s) built per group on DVE from an f16 identity.
  - AV accumulates m-chunks into one PSUM tile per 4-group block; the
    output projection + store for rows 0:128 completes mid-exp-phase,
    leaving only the last block's chain on the tail.
  - All heavy matmuls are f32r with >=256 output columns (1 cycle/row);
    AV/transposes run in f16.  A PE p-state warmup (dummy matmuls at
    t~0) and an early dummy Silu (pulls the ACT table load into the
    initial DMA wait) trim the startup; DMAs are spread across the
    SP/HWDGE and Pool/SWDGE queues with critical constants first.
"""
import sys

sys.path.insert(0, "/opt/trn_rl_repo")

import numpy as np
import concourse.bacc as bacc
import concourse.mybir as mybir
import concourse.tile as tile
from concourse.bass_utils import run_bass_kernel_spmd

F32 = mybir.dt.float32
F32R = mybir.dt.float32r
F16 = mybir.dt.float16
AF = mybir.ActivationFunctionType

B, N, C, H, DG, HID = 4, 512, 256, 8, 6, 16
D = C // H          # 32
NP = N // 2         # 256 query rows per core
NG = NP // 16       # 16 groups of 16 rows
MC0 = 256           # main key chunk
NEG = np.float32(-1e38)
AV_DT = F16


def _build(nc_mod, m_pad):
    nc = nc_mod
    M = m_pad
    MC1 = M - MC0               # tail key chunk (16 for this seed)
    assert MC1 >= 1 and 1024 % MC1 == 0 and NP % (1024 // MC1) == 0
    C1R = 1024 // MC1           # rows per c1 step (64)
    C1S = NP // C1R             # c1 steps (4)
    SUB = 512 // MC1            # rows per c1 matmul (32)
    MT = [(t, min(128, M - 128 * t)) for t in range((M + 127) // 128)]

    # ---------------- I/O ----------------
    gt = nc.declare_dram_parameter("gt", [DG, NP, MC0], F32, isOutput=False)
    gtc1 = nc.declare_dram_parameter("gtc1", [DG, NP, MC1], F32, isOutput=False)
    ctq = nc.declare_dram_parameter("ctq", [C, NP], F32, isOutput=False)
    ctk = nc.declare_dram_parameter("ctk", [C, M], F32, isOutput=False)
    wq = nc.declare_dram_parameter("wq", [C, C], F32, isOutput=False)
    wk = nc.declare_dram_parameter("wk", [C, C], F32, isOutput=False)
    win = nc.declare_dram_parameter("win", [C, C], F32, isOutput=False)
    wout = nc.declare_dram_parameter("wout", [C, C], F32, isOutput=False)
    bq = nc.declare_dram_parameter("bq", [1, C], F32, isOutput=False)
    bk = nc.declare_dram_parameter("bk", [1, C], F32, isOutput=False)
    bin_ = nc.declare_dram_parameter("bin", [1, C], F32, isOutput=False)
    bout = nc.declare_dram_parameter("bout", [1, C], F32, isOutput=False)
    w1 = nc.declare_dram_parameter("w1", [DG, 128], F32, isOutput=False)
    w2 = nc.declare_dram_parameter("w2", [128, 128], F32, isOutput=False)
    w3 = nc.declare_dram_parameter("w3", [128, 8], F32, isOutput=False)
    b1 = nc.declare_dram_parameter("b1", [128, 1], F32, isOutput=False)
    b2 = nc.declare_dram_parameter("b2", [128, 1], F32, isOutput=False)
    b3 = nc.declare_dram_parameter("b3", [128, 1], F32, isOutput=False)
    mb = nc.declare_dram_parameter("mb", [1, M], F32, isOutput=False)
    onesc = nc.declare_dram_parameter("onesc", [1, 128], F32, isOutput=False)
    zeros = nc.declare_dram_parameter("zeros", [128, 128], F32, isOutput=False)
    identh = nc.declare_dram_parameter("identh", [128, 128], AV_DT,
                                       isOutput=False)
    out = nc.declare_dram_parameter("out", [NP, C], F32, isOutput=True)

    with tile.TileContext(nc) as tc:
        import contextlib
        with contextlib.ExitStack() as ctx:
            cst = ctx.enter_context(tc.tile_pool(name="cst", bufs=1))
            big = ctx.enter_context(tc.tile_pool(name="big", bufs=1))
            gtp = ctx.enter_context(tc.tile_pool(name="gtp", bufs=2))
            xp = ctx.enter_context(tc.tile_pool(name="xp", bufs=2))
            smp = ctx.enter_context(tc.tile_pool(name="smp", bufs=3))
            pmm = ctx.enter_context(tc.tile_pool(name="pmm", bufs=2,
                                                 space="PSUM"))
            pl3 = ctx.enter_context(tc.tile_pool(name="pl3", bufs=1,
                                                 space="PSUM"))
            pms = ctx.enter_context(tc.tile_pool(name="pms", bufs=1,
                                                 space="PSUM"))

            # ---- critical-path constants on the SP/HWDGE queue ----
            gtc1_sb = cst.tile([DG, NP, MC1], F32R, tag="gtc1")
            nc.sync.dma_start(out=gtc1_sb, in_=gtc1[:, :, :].bitcast(F32R))
            w1_sb = cst.tile([DG, 128], F32R, tag="w1")
            nc.sync.dma_start(out=w1_sb, in_=w1[:, :].bitcast(F32R))
            b1_sb = cst.tile([128, 1], F32, tag="b1")
            nc.sync.dma_start(out=b1_sb, in_=b1[:, :])
            w2_sb = cst.tile([128, 128], F32R, tag="w2")
            nc.sync.dma_start(out=w2_sb, in_=w2[:, :].bitcast(F32R))
            b2_sb = cst.tile([128, 1], F32, tag="b2")
            nc.sync.dma_start(out=b2_sb, in_=b2[:, :])
            b3_sb = cst.tile([128, 1], F32, tag="b3")
            nc.sync.dma_start(out=b3_sb, in_=b3[:, :])

            gt_tiles = {}

            def fetch_gt(g):
                t = gtp.tile([DG, 16, MC0], F32R, tag="gt", name=f"gt{g}")
                nc.sync.dma_start(
                    out=t, in_=gt[:, 16 * g:16 * (g + 1), :].bitcast(F32R))
                gt_tiles[g] = t

            fetch_gt(0)
            fetch_gt(1)

            # ---- early constants on the DVE queue ----
            onr = cst.tile([1, 512], F32R, tag="onr")
            nc.vector.memset(onr.bitcast(F32), 1.0)
            # dummy silu: pulls the Silu table load to t~0 (hidden in the
            # initial DMA wait) so the first real silu pays no load
            scrap = cst.tile([1, 1], F32, tag="scrap")
            nc.scalar.activation(out=scrap, in_=b1_sb[:1, :], func=AF.Silu,
                                 bias=b1_sb[:1, :], scale=1.0)
            # PE p-state warmup: tiny dummy matmuls start the clock-ramp
            # timer while the critical DMAs are still in flight
            pwarm = pms.tile([128, 512], F32, tag="ms", name="pwarm")
            for _ in range(3):
                nc.tensor.matmul(pwarm[:, :128], onr[:, :128],
                                 onr[:, :128], start=True, stop=True)
            wq_sb, wk_sb, ctq_sb, ctk_sb = [], [], [], []
            for ci in range(2):
                t = cst.tile([128, C], F32R, tag=f"wq{ci}", name=f"wq{ci}")
                nc.sync.dma_start(
                    out=t, in_=wq[128 * ci:128 * (ci + 1), :].bitcast(F32R))
                wq_sb.append(t)
                t = cst.tile([128, NP], F32R, tag=f"cq{ci}", name=f"cq{ci}")
                nc.sync.dma_start(
                    out=t, in_=ctq[128 * ci:128 * (ci + 1), :].bitcast(F32R))
                ctq_sb.append(t)
            bq_sb = cst.tile([1, C], F32R, tag="bq")
            nc.sync.dma_start(out=bq_sb, in_=bq[:, :].bitcast(F32R))

            # ---- Pool (SWDGE) queue: w3 variant sources ----
            zs = cst.tile([128, 128], F32R, tag="zs")
            nc.gpsimd.dma_start(out=zs, in_=zeros[:, :].bitcast(F32R))
            w3c = cst.tile([128, 8], F32R, tag="w3c")
            nc.gpsimd.dma_start(out=w3c, in_=w3[:, :].bitcast(F32R))
            w3_sb = [cst.tile([128, 128], F32R, tag=f"w3{j}", name=f"w3{j}")
                     for j in range(16)]

            def build_w3(j):
                nc.gpsimd.tensor_copy(out=w3_sb[j], in_=zs)
                nc.gpsimd.tensor_copy(out=w3_sb[j][:, 8 * j:8 * j + 8],
                                      in_=w3c)

            # ---- persistent SBUF tensors ----
            qt_sb = [big.tile([128, NP], F32R, tag=f"qt{i}", name=f"qt{i}")
                     for i in range(2)]
            kt_sb = [big.tile([128, M], F32R, tag=f"kt{i}", name=f"kt{i}")
                     for i in range(2)]
            qm_all = [big.tile([128, NP, 8], F32R, tag=f"qm{i}",
                               name=f"qm{i}") for i in range(2)]
            x2c1 = big.tile([128, NP, MC1], F32R, tag="x2c1")
            aloc = big.tile([128, NG, M], F32, tag="aloc")
            pre = big.tile([128, NG, M], F32, tag="pre")
            attT = big.tile([128, len(MT), NG, 16, 8], AV_DT, tag="attT")
            v_sb = [big.tile([128, C], AV_DT, tag=f"v{t}", name=f"v{t}")
                    for t, _ in MT]
            win_sb, wout_sb = [], []
            misc = {}

            # ---- deferred-prologue emitters ----
            def emit_dve_batch2():
                for ci in range(2):
                    t = cst.tile([128, C], F32R, tag=f"wk{ci}", name=f"wk{ci}")
                    nc.sync.dma_start(
                        out=t, in_=wk[128 * ci:128 * (ci + 1), :].bitcast(F32R))
                    wk_sb.append(t)
                    t = cst.tile([128, M], F32R, tag=f"ck{ci}", name=f"ck{ci}")
                    nc.sync.dma_start(
                        out=t, in_=ctk[128 * ci:128 * (ci + 1), :].bitcast(F32R))
                    ctk_sb.append(t)
                t = cst.tile([1, C], F32R, tag="bk", name="bk_sb")
                nc.sync.dma_start(out=t, in_=bk[:, :].bitcast(F32R))
                misc["bk"] = t
                t = cst.tile([1, M], F32R, tag="mb", name="mb_sb")
                nc.sync.dma_start(out=t, in_=mb[:, :].bitcast(F32R))
                misc["mb"] = t
                t = cst.tile([1, 128], F32R, tag="onc", name="onc_sb")
                nc.sync.dma_start(out=t, in_=onesc[:, :].bitcast(F32R))
                misc["onc"] = t

            def proj_T(dst, w_tiles, b_row, rhs_tiles, nfree, nm, ct):
                p = pms.tile([128, 512], F32, tag="ms",
                             name=f"pj_{nm}{ct}")
                for ci in range(2):
                    nc.tensor.matmul(
                        p[:, :nfree],
                        w_tiles[ci][:, 128 * ct:128 * (ct + 1)],
                        rhs_tiles[ci][:, :nfree],
                        start=(ci == 0), stop=False)
                nc.tensor.matmul(
                    p[:, :nfree], b_row[:, 128 * ct:128 * (ct + 1)],
                    onr[:, :nfree], start=False, stop=True)
                nc.vector.tensor_copy(out=dst[ct], in_=p[:, :nfree])

            def emit_qt(ct):
                proj_T(qt_sb, wq_sb, bq_sb, ctq_sb, NP, "q", ct)

            def emit_kt(ct):
                proj_T(kt_sb, wk_sb, misc["bk"], ctk_sb, M, "k", ct)

            def emit_qm_zero():
                nc.gpsimd.memset(qm_all[0].bitcast(F32), 0.0)
                nc.gpsimd.memset(qm_all[1].bitcast(F32), 0.0)

            def emit_qm_fill():
                for h in range(H):
                    a = 32 * (h % 4)
                    nc.vector.tensor_copy(
                        out=qm_all[h // 4][a:a + 32, :, h],
                        in_=qt_sb[h // 4][a:a + 32, :])

            def emit_v_consts():
                for ci in range(2):
                    t = cst.tile([128, C], F32R, tag=f"wi{ci}", name=f"wi{ci}")
                    nc.gpsimd.dma_start(
                        out=t, in_=win[128 * ci:128 * (ci + 1), :].bitcast(F32R))
                    win_sb.append(t)
                t = cst.tile([1, C], F32R, tag="bi", name="bi_sb")
                nc.gpsimd.dma_start(out=t, in_=bin_[:, :].bitcast(F32R))
                misc["bi"] = t

            def emit_v(t_, msz):
                p = pms.tile([128, 512], F32, tag="ms", name=f"pv{t_}")
                for ci in range(2):
                    nc.tensor.matmul(
                        p[:msz, :C],
                        ctk_sb[ci][:, 128 * t_:128 * t_ + msz],
                        win_sb[ci], start=(ci == 0), stop=False)
                nc.tensor.matmul(
                    p[:msz, :C], misc["onc"][:, :msz],
                    misc["bi"], start=False, stop=True)
                nc.vector.tensor_copy(out=v_sb[t_][:msz, :],
                                      in_=p[:msz, :C])

            def emit_tail_consts():
                for ci in range(2):
                    t = cst.tile([128, C], F32R, tag=f"wo{ci}", name=f"wo{ci}")
                    nc.gpsimd.dma_start(
                        out=t,
                        in_=wout[128 * ci:128 * (ci + 1), :].bitcast(F32R))
                    wout_sb.append(t)
                t = cst.tile([1, C], F32R, tag="bo", name="bo_sb")
                nc.gpsimd.dma_start(out=t, in_=bout[:, :].bitcast(F32R))
                misc["bo"] = t
                t = cst.tile([128, 128], AV_DT, tag="idh", name="idh_sb")
                nc.gpsimd.dma_start(out=t, in_=identh[:, :])
                misc["idh"] = t

            # ---------------- c1 prephase (keys [256, m_pad)) -----------
            x2c1v = x2c1.rearrange("p (g r) k -> p g r k", g=NG)
            x1cs = {}

            def emit_l2c1(s):
                p2c = pmm.tile([128, C1R, MC1], F32, tag="mm",
                               name=f"p2c{s}")
                for j in range(0, C1R, SUB):
                    nc.tensor.matmul(p2c[:, j:j + SUB, :], w2_sb,
                                     x1cs[s][:, j:j + SUB, :],
                                     start=True, stop=True)
                nc.scalar.activation(
                    out=x2c1[:, s * C1R:(s + 1) * C1R, :], in_=p2c,
                    func=AF.Silu, bias=b2_sb, scale=1.0)

            for s in range(C1S):
                if s == 0:
                    emit_dve_batch2()
                for j in range(4):
                    build_w3(4 * s + j)
                p1c = pmm.tile([128, C1R, MC1], F32, tag="mm",
                               name=f"p1c{s}")
                for j in range(0, C1R, SUB):
                    nc.tensor.matmul(
                        p1c[:, j:j + SUB, :], w1_sb,
                        gtc1_sb[:, s * C1R + j:s * C1R + j + SUB, :],
                        start=True, stop=True)
                x1c = xp.tile([128, C1R, MC1], F32R, tag="x1",
                              name=f"x1c{s}")
                nc.scalar.activation(out=x1c, in_=p1c, func=AF.Silu,
                                     bias=b1_sb, scale=1.0)
                x1cs[s] = x1c
                if s >= 1:
                    emit_l2c1(s - 1)
                if s == 1:
                    emit_qt(0)
                    emit_v_consts()
                if s == 2:
                    emit_qt(1)
                if s == 3:
                    emit_kt(0)
                    emit_qm_zero()
                    emit_v(*MT[0])
            emit_l2c1(C1S - 1)

            def emit_l3c1():
                pc1 = pl3.tile([128, 512], F32, tag="l3", name="pc1")
                pc1v = pc1[:, :NG * MC1].rearrange("p (g k) -> p g k", g=NG)
                for r in range(16):
                    nc.tensor.matmul(pc1v, w3_sb[r], x2c1v[:, :, r, :],
                                     start=(r == 0), stop=(r == 15))
                nc.scalar.activation(out=aloc[:, :, MC0:], in_=pc1v,
                                     func=AF.Silu, bias=b3_sb, scale=1.0)

            # ---------------- main pipeline: 64 steps of 4 rows ----------
            deferred = {
                0: [lambda: emit_kt(1), lambda: emit_v(*MT[1])],
                1: ([emit_qm_fill, lambda: emit_v(*MT[2])] if len(MT) > 2
                    else [emit_qm_fill]),
                4: [emit_tail_consts],
            }
            STEPS = [(g, r0, nr) for g in range(NG)
                     for r0, nr in ((0, 6), (6, 6), (12, 4))]
            NS = len(STEPS)
            p3s = {}
            pafs = {}
            x1s = {}
            x2s = {}

            def emit_l2(k):
                # L2 of step k, one step behind L1 (keeps ACT off the
                # L2 latency chain)
                gk, rk, nk = STEPS[k]
                p2 = pmm.tile([128, 6, MC0], F32, tag="mm", name=f"p2_{k}")
                for j in range(nk):
                    nc.tensor.matmul(p2[:, j, :], w2_sb, x1s[k][:, j, :],
                                     start=True, stop=True)
                x2 = xp.tile([128, 6, MC0], F32R, tag="x2", name=f"x2_{k}")
                nc.scalar.activation(out=x2[:, :nk, :], in_=p2[:, :nk, :],
                                     func=AF.Silu, bias=b2_sb, scale=1.0)
                x2s[k] = x2
                del x1s[k]

            def emit_l3(k):
                # L3 of a group pair accumulates into one bank: even group
                # in [0:256], odd group in [256:512]; one silu3 covers both.
                gk, rk, nk = STEPS[k]
                q = gk // 2
                if rk == 0 and gk % 2 == 0:
                    p3s[q] = pl3.tile([128, 2, MC0], F32, tag="l3",
                                      name=f"p3_{q}")
                for j in range(nk):
                    nc.tensor.matmul(p3s[q][:, gk % 2, :], w3_sb[rk + j],
                                     x2s[k][:, j, :],
                                     start=(rk + j == 0), stop=(rk + j == 15))
                del x2s[k]
                if rk + nk == 16:
                    # pre := paf as soon as A_feat lands (per group)
                    nc.vector.tensor_copy(out=pre[:, gk, :],
                                          in_=pafs.pop(gk)[:, :M])
                    if gk % 2 == 1:
                        nc.scalar.activation(
                            out=aloc[:, 2 * q:2 * q + 2, :MC0],
                            in_=p3s[q], func=AF.Silu, bias=b3_sb, scale=1.0)
                        nc.vector.tensor_add(
                            pre[:, 2 * q:2 * q + 2, :],
                            pre[:, 2 * q:2 * q + 2, :],
                            aloc[:, 2 * q:2 * q + 2, :])

            for i in range(NS):
                g, r0, nr = STEPS[i]
                if r0 == 0 and g + 2 < NG:
                    fetch_gt(g + 2)
                p1 = pmm.tile([128, 6, MC0], F32, tag="mm", name=f"p1_{i}")
                for j in range(nr):
                    nc.tensor.matmul(p1[:, j, :], w1_sb,
                                     gt_tiles[g][:, r0 + j, :],
                                     start=True, stop=True)
                x1 = xp.tile([128, 6, MC0], F32R, tag="x1", name=f"x1_{i}")
                nc.scalar.activation(out=x1[:, :nr, :], in_=p1[:, :nr, :],
                                     func=AF.Silu, bias=b1_sb, scale=1.0)
                x1s[i] = x1
                if i == 0:
                    emit_l3c1()
                if i >= 1:
                    emit_l2(i - 1)
                if i >= 2:
                    emit_l3(i - 2)
                if (r0 == 6 and g > 0) or (r0 == 12 and g == 0):
                    paf = pms.tile([128, 512], F32, tag="ms", name=f"paf{g}")
                    nc.tensor.matmul(paf[:, :M],
                                     qm_all[0][:, 16 * g:16 * g + 16, :],
                                     kt_sb[0], start=True, stop=False)
                    nc.tensor.matmul(paf[:, :M],
                                     qm_all[1][:, 16 * g:16 * g + 16, :],
                                     kt_sb[1], start=False, stop=False)
                    nc.tensor.matmul(paf[:, :M], misc["onc"], misc["mb"],
                                     start=False, stop=True)
                    pafs[g] = paf
                for fn in deferred.get(i, ()):
                    fn()
            emit_l2(NS - 1)
            emit_l3(NS - 2)
            emit_l3(NS - 1)

            # ---------------- P2: exp softmax + scaled transpose ---------
            # bneg4 = -4.0, built from the LAST silu3 output so the Tile
            # scheduler cannot hoist any Exp (different ACT table) into the
            # Silu stream.
            bneg4 = cst.tile([128, 1], F32, tag="bneg4")
            nc.scalar.activation(out=bneg4, in_=aloc[:, NG - 1, :1],
                                 func=AF.Copy, bias=-4.0, scale=0.0)
            pav = pmm.tile([128, 4, MC0], F32, tag="mm", name="pav")
            pavv = pav.rearrange("p a b -> p (a b)")[:, :2 * C].rearrange(
                "p (a b) -> p a b", a=2)

            avf = smp.tile([128, 2, C], F32R, tag="avf")
            po_t = {}

            def emit_av(g_lo, g_hi):
                w = 16 * (g_hi - g_lo + 1)
                for h in range(H):
                    for t_, msz in MT:
                        nc.tensor.matmul(
                            pavv[32 * (h % 4):32 * (h % 4) + 32, h // 4,
                                 16 * g_lo:16 * g_lo + w],
                            v_sb[t_][:msz, 32 * h:32 * h + 32],
                            attT[:msz, t_, g_lo:g_hi + 1, :, h],
                            start=(t_ == 0),
                            stop=(t_ == len(MT) - 1),
                            tile_position=(0, 32 * (h % 4)))

            def emit_avf(nt, eng):
                eng(out=avf[:, :, 128 * nt:128 * (nt + 1)],
                    in_=pavv[:, :, 128 * nt:128 * (nt + 1)])

            def emit_po(nt):
                po = pmm.tile([128, 6, MC0], F32, tag="mm", name=f"po{nt}")
                for ci in range(2):
                    nc.tensor.matmul(
                        po[:, 0, :C], avf[:, ci, 128 * nt:128 * (nt + 1)],
                        wout_sb[ci], start=(ci == 0), stop=False)
                nc.tensor.matmul(po[:, 0, :C], misc["onc"],
                                 misc["bo"], start=False, stop=True)
                po_t[nt] = po

            def emit_osb(nt, eng):
                o_sb = smp.tile([128, C], F32, tag="osb", name=f"osb{nt}")
                eng(out=o_sb, in_=po_t[nt][:, 0, :C])
                nc.sync.dma_start(out=out[128 * nt:128 * (nt + 1), :],
                                  in_=o_sb)

            for g in range(NG):
                att = smp.tile([128, M], AV_DT, tag="att", name=f"att{g}",
                               bufs=6)
                sm = smp.tile([128, 1], F32, tag="sm", name=f"sm{g}", bufs=6)
                nc.scalar.activation(out=att, in_=pre[:, g, :], func=AF.Exp,
                                     bias=bneg4, scale=1.0, accum_out=sm)
                rc = smp.tile([128, 1], F32, tag="rc", name=f"rc{g}", bufs=6)
                nc.vector.reciprocal(out=rc, in_=sm)
                dg_t = smp.tile([128, 128], AV_DT, tag="dg", name=f"dg{g}",
                                bufs=6)
                nc.vector.tensor_scalar_mul(dg_t, misc["idh"], rc)
                pT = (pms if g % 2 == 0 else pl3).tile(
                    [128, 512], F32, tag=("ms" if g % 2 == 0 else "l3"),
                    name=f"pT{g}")
                pT3 = pT[:, :len(MT) * 128].rearrange("p (t q) -> p t q",
                                                      t=len(MT))
                pT3r = pT[:, :len(MT) * 128].rearrange(
                    "p (t r h) -> p t r h", t=len(MT), r=16)
                for t_, msz in MT:
                    nc.tensor.matmul(pT3[:msz, t_, :],
                                     att[:, 128 * t_:128 * t_ + msz],
                                     dg_t, start=True, stop=True)
                if g % 4 == 1:
                    nc.scalar.copy(out=attT[:, :, g, :, :], in_=pT3r)
                else:
                    nc.vector.tensor_copy(out=attT[:, :, g, :, :], in_=pT3r)
                if g == 3:
                    emit_av(0, 3)
                elif g == 7:
                    emit_av(4, 7)
                elif g == 8:
                    emit_avf(0, nc.vector.tensor_copy)
                elif g == 9:
                    emit_po(0)
                elif g == 10:
                    emit_osb(0, nc.vector.tensor_copy)
                elif g == 11:
                    emit_av(8, 11)
                elif g == 14:
                    emit_av(12, 14)
                elif g == 15:
                    emit_av(15, 15)

            # ---------------- P3: finish rows 128..255 -------------------
            emit_avf(1, nc.vector.tensor_copy)
            emit_po(1)
            emit_osb(1, nc.scalar.copy)

    nc.finalize()
    return nc


_CACHE = {}


def _get_nc(m_pad):
    if m_pad not in _CACHE:
        _CACHE[m_pad] = _build(bacc.Bacc(None, target_bir_lowering=False), m_pad)
    return _CACHE[m_pad]


def prepare(inputs):
    """Host-side sharding/packing. Returns (nc, in_maps, assemble)."""
    pg = np.asarray(inputs["pairwise_g"], np.float32)
    cf = np.asarray(inputs["coset_functions"], np.float32)
    mask = np.asarray(inputs["mask"])
    idxs = [np.where(mask[b])[0] for b in range(B)]
    maxc = max(len(ix) for ix in idxs)
    m_pad = max(MC0 + 16, -(-maxc // 16) * 16)

    w1a = np.ascontiguousarray(
        np.asarray(inputs["loc_w1"], np.float32).transpose(1, 0, 2).reshape(
            DG, 128))
    w2b = np.zeros((128, 128), np.float32)
    lw2 = np.asarray(inputs["loc_w2"], np.float32)
    for h in range(H):
        w2b[16 * h:16 * (h + 1), 16 * h:16 * (h + 1)] = lw2[h]
    lw3 = np.asarray(inputs["loc_w3"], np.float32)
    w3p = np.zeros((128, 8), np.float32)
    for h in range(H):
        w3p[16 * h:16 * (h + 1), h] = lw3[h, :, 0]
    b1v = np.asarray(inputs["loc_b1"], np.float32).reshape(128, 1)
    b2v = np.asarray(inputs["loc_b2"], np.float32).reshape(128, 1)
    b3v = np.tile(np.asarray(inputs["loc_b3"], np.float32).reshape(8), 16)
    b3v = b3v.reshape(128, 1)

    idh = np.eye(128, dtype=np.float16)

    common = {
        "wq": np.asarray(inputs["fc_q_w"], np.float32) / np.float32(16.0),
        "wk": np.asarray(inputs["fc_k_w"], np.float32),
        "win": np.asarray(inputs["in_w"], np.float32),
        "wout": np.asarray(inputs["out_w"], np.float32),
        "bq": (np.asarray(inputs["fc_q_b"], np.float32) / np.float32(16.0)
               ).reshape(1, C),
        "bk": np.asarray(inputs["fc_k_b"], np.float32).reshape(1, C),
        "bin": np.asarray(inputs["in_b"], np.float32).reshape(1, C),
        "bout": np.asarray(inputs["out_b"], np.float32).reshape(1, C),
        "w1": w1a, "w2": w2b, "w3": w3p,
        "b1": b1v, "b2": b2v, "b3": b3v,
        "onesc": np.ones((1, 128), np.float32),
        "zeros": np.zeros((128, 128), np.float32),
        "identh": idh,
    }
    common = {k: np.ascontiguousarray(v) for k, v in common.items()}

    in_maps = []
    for core in range(8):
        b, nh = core // 2, core % 2
        ix = idxs[b]
        cnt = len(ix)
        n0 = NP * nh
        gtb = np.zeros((DG, NP, m_pad), np.float32)
        gtb[:, :, :cnt] = pg[b, n0:n0 + NP][:, ix, :].transpose(2, 0, 1)
        ctkb = np.zeros((C, m_pad), np.float32)
        ctkb[:, :cnt] = cf[b, ix, :].T
        mbv = np.zeros((1, m_pad), np.float32)
        mbv[0, cnt:] = NEG
        im = dict(common)
        im["gt"] = np.ascontiguousarray(gtb[:, :, :MC0])
        im["gtc1"] = np.ascontiguousarray(gtb[:, :, MC0:])
        im["ctq"] = np.ascontiguousarray(cf[b, n0:n0 + NP, :].T)
        im["ctk"] = ctkb
        im["mb"] = mbv
        in_maps.append(im)

    def assemble(results):
        o = np.empty((B, N, C), np.float32)
        for core in range(8):
            b, nh = core // 2, core % 2
            o[b, NP * nh:NP * (nh + 1), :] = results[core]["out"]
        return o

    return _get_nc(m_pad), in_maps, assemble


def kernel(**inputs) -> np.ndarray:
    nc, in_maps, assemble = prepare(inputs)
    res = run_bass_kernel_spmd(nc, in_maps, list(range(8)))
    return assemble(res.results)
